# revision 44
# baseline (speedup 1.0000x reference)
"""Multi-head self-attention (B=2, S=2048, D=1024, H=16, causal) on 8 TRN2 cores.

Sharding: core c handles batch b=c//4 and head-group g=c%4 (4 heads each).
Host pre-transposes x and the weight slices so the kernel never needs an
on-chip transpose, and pre-converts them to bf16 (PE streams run at
1 cycle/row in bf16; DMA halves):
  xT   [1024, 2048] = x[b].T
  wqT/wkT/wvT [1024, 256] = W.T[:, g*256:(g+1)*256]
  woT  [256, 1024] = Wo[:, g*256:(g+1)*256].T
The kernel writes bf16 partial outputs; host sums the 4 per-group partials
per batch in fp32 at the end.

On-chip dataflow per core:
  qT/kT [256, 2048] (head dim on partitions), v [2048, 4*65] (with a ones
  column appended per head so the PV matmul also accumulates the softmax
  denominator in psum row 64).  Scores are computed transposed
  (scoresT[j, i]) so softmax needs no transpose at all; no max-subtraction
  (scores are O(+-6), exp is safe in fp32).

Performance notes (the PE tensor engine only reaches its ramped 2.4 GHz
p-state after ~3us of *continuous* work; any idle resets it to 1.2 GHz, so
everything is built around never letting the PE wait):
  - Attention runs a lag-2 software pipeline: scores(jc) are emitted two
    chunks ahead of PV(jc), so the PE has ~1.3us of score matmuls in its
    queue while the Scalar engine exponentiates a chunk.
  - Causal trimming: for the 4 diagonal j-chunks of each query block the
    scores / exp / PV are restricted to the query range [o*128, 512) that
    can actually attend to that chunk; the causal mask reduces to a single
    [128,128] lower-triangular multiply per diagonal chunk.
  - All PSUM tiles are one 2KB bank: a 6-deep "mm" pool (scores, QKV
    projections, out-proj) + 2 PV accumulators.
  - The output projection of block Q-1 is interleaved two sub-blocks at a
    time into the starts of block Q's two head-pair groups, filling the
    PE while the softmax denominators of the previous group are applied.
  - The softmax 1/denominator is broadcast across the 64 head-dim
    partitions on the (otherwise idle) GpSimd engine.
  - xT is DMAed in four column-quarters and the first projection group
    only needs the first quarter; DMA descriptor generation (~0.6us per
    transfer, serial per issuing queue) is kept off the critical path.
"""

import os
import sys

sys.path.insert(0, "/opt/trn_rl_repo")
os.environ.setdefault("MYCRO_LOCAL_CACHE", "1")

import numpy as np
import ml_dtypes

import concourse.bacc as bacc
import concourse.bass as bass
import concourse.mybir as mybir
import concourse.tile as tile
from concourse import bass_utils

# The agent image's antenv lacks axon_hooks, so bass_utils' trace path dies on
# import.  Register a shim module that lazily builds the ctypes NTFF hook.
if "antenv.axon_hooks" not in sys.modules:
    import types

    _shim = types.ModuleType("antenv.axon_hooks")
    _shim._HOOK = None

    def _set_hook(hook, _m=_shim):
        _m._HOOK = hook

    def _get_hook(_m=_shim):
        if _m._HOOK is None:
            try:
                from trn_agent_boot.trn_boot import _ntff_profile_via_ctypes

                _m._HOOK = _ntff_profile_via_ctypes("/opt/axon/libaxon_pjrt.so")
            except Exception:
                _m._HOOK = None
        return _m._HOOK

    _shim.set_axon_ntff_profile_hook = _set_hook
    _shim.get_axon_ntff_profile_hook = _get_hook
    sys.modules["antenv.axon_hooks"] = _shim

B, S, D, H = 2, 2048, 1024, 16
DK = 64                      # head dim
HC = 4                       # heads per core
GC = HC * DK                 # 256 cols per head-group
N_CORES = 8
SCALE = 1.0 / np.sqrt(DK)    # 0.125

F32 = mybir.dt.float32
BF16 = mybir.dt.bfloat16
NP_BF16 = ml_dtypes.bfloat16

TRACE = False
LAST_RESULTS = None


def build_bass():
    nc = bacc.Bacc("TRN2", target_bir_lowering=False, debug=False)

    # host-repacked layouts: partition-major with long contiguous lines
    # (8KB for x, 4KB for weights) — short (<=1KB) per-partition DMA lines
    # halve the effective HBM bandwidth
    xT_d = nc.dram_tensor("xT", [128, 4, 8, 512], BF16, kind="ExternalInput")
    wqT_d = nc.dram_tensor("wqT", [128, 8, GC], BF16, kind="ExternalInput")
    wkT_d = nc.dram_tensor("wkT", [128, 8, GC], BF16, kind="ExternalInput")
    wvT_d = nc.dram_tensor("wvT", [128, 8, GC], BF16, kind="ExternalInput")
    woT_d = nc.dram_tensor("woT", [128, 2, D], BF16, kind="ExternalInput")
    mask_d = nc.dram_tensor("mask", [128, 128], BF16, kind="ExternalInput")
    out_d = nc.dram_tensor("out", [S, D], BF16, kind="ExternalOutput")

    EXP = mybir.ActivationFunctionType.Exp

    with tile.TileContext(nc) as tc:
        with (
            nc.allow_low_precision(reason="bf16 matmuls, fp32 psum accumulate"),
            tc.tile_pool(name="const", bufs=1) as const,
            tc.tile_pool(name="work", bufs=6) as work,
            tc.tile_pool(name="apool", bufs=2) as apool,
            tc.tile_pool(name="opool", bufs=2) as opool,
            tc.tile_pool(name="rpool", bufs=2) as rpool,
            tc.tile_pool(name="pmm", bufs=4, space="PSUM") as pmm,
            tc.tile_pool(name="psout", bufs=4, space="PSUM") as psout,
        ):
            # ---- load inputs -------------------------------------------------
            # one tile per s-quarter with exactly one DMA each: a reader of
            # quarter q then only waits for quarter q's transfer (a single
            # shared tile would make every reader wait for the whole 4MB)
            xqs = [const.tile([128, 8, 512], BF16, name=f"xq{q}")
                   for q in range(4)]
            # Concurrent DMAs share HBM bandwidth (completion is NOT FIFO),
            # so only the first projection wave's data (wq+wk+wv+quarter 0,
            # ~1.4MB) is put in flight up front.  Quarters 1-3 are released
            # from the scalar queue behind a dummy op that depends on the
            # previous wave's first result, keeping each quarter's transfer
            # nearly exclusive while the PE computes the wave before it.
            nc.sync.dma_start(xqs[0][:], xT_d[:, 0])
            wq = const.tile([128, 8, GC], BF16)
            nc.sync.dma_start(wq[:], wqT_d[:])
            wk = const.tile([128, 8, GC], BF16)
            nc.gpsimd.dma_start(wk[:], wkT_d[:])
            wv = const.tile([128, 8, GC], BF16)
            nc.gpsimd.dma_start(wv[:], wvT_d[:])
            wo = const.tile([128, 2, D], BF16)
            maskt = const.tile([128, 128], BF16)

            ones_b = const.tile([128, 64], BF16)
            nc.vector.memset(ones_b[:], 1.0)
            ones64 = const.tile([1, 64], BF16)
            nc.vector.tensor_copy(ones64[:], ones_b[0:1, :])

            # ---- projections -------------------------------------------------
            # qT/kT: per (head-pair mo, s-half sbh) tiles [128, 1024] so the
            # attention phase can start before all projections finish
            qts = [[const.tile([128, 1024], BF16, name=f"q{m}{s}")
                    for s in range(2)] for m in range(2)]
            kts = [[const.tile([128, 1024], BF16, name=f"k{m}{s}")
                    for s in range(2)] for m in range(2)]
            # v: per j-chunk tiles; per head: 64 value cols + 1 ones col
            vts = []
            for io in range(16):
                vt = const.tile([128, HC * 65], BF16, name=f"v{io}")
                nc.vector.tensor_copy(
                    vt.rearrange("p (h u) -> p h u", u=65)[:, :, 64],
                    ones_b[:, 0:4],
                )
                vts.append(vt)

            # s-quarter outer: each quarter's groups (q, k, and v) only gate
            # on that quarter's slice of the xT DMA, so the PE paces along
            # right behind the input stream
            for sb in range(4):
                for w_sb, dst in ((wq, qts), (wk, kts)):
                    for mo in range(2):
                        ps = pmm.tile([128, 512], F32, tag="mm")
                        for ko in range(8):
                            nc.tensor.matmul(
                                ps[:],
                                (w_sb[:, ko, mo * 128:(mo + 1) * 128]),
                                (xqs[sb][:, ko, :]),
                                start=(ko == 0),
                                stop=(ko == 7),
                                skip_group_check=True,
                            )
                        nc.vector.tensor_copy(
                            dst[mo][sb // 2][:, (sb % 2) * 512:(sb % 2 + 1) * 512],
                            ps[:],
                        )
                        if w_sb is wq and mo == 0 and sb < 3:
                            # release the next quarter only once this wave's
                            # first copy lands: the tiny copy into the target
                            # tile gives the deferred DMA a write-after-write
                            # dependency the scheduler cannot hoist past
                            nc.vector.tensor_copy(
                                xqs[sb + 1][0:1, 0:1, 0:8],
                                dst[mo][sb // 2][0:1, (sb % 2) * 512:
                                                 (sb % 2) * 512 + 8],
                            )
                            nc.sync.dma_start(
                                xqs[sb + 1][:], xT_d[:, sb + 1]
                            )
                            if sb == 0:
                                nc.sync.dma_start(wo[:], woT_d[:])
                                nc.sync.dma_start(maskt[:], mask_d[:])
                for io in range(4 * sb, 4 * sb + 4):
                    ps = pmm.tile([128, 256], F32, tag="mm")
                    for ko in range(8):
                        nc.tensor.matmul(
                            ps[:],
                            (xqs[sb][:, ko, (io % 4) * 128:(io % 4 + 1) * 128]),
                            (wv[:, ko, :]),
                            start=(ko == 0),
                            stop=(ko == 7),
                        )
                    nc.vector.tensor_copy(
                        vts[io].rearrange("p (h u) -> p h u", u=65)[:, :, 0:64],
                        ps.rearrange("p (h e) -> p h e", e=64),
                    )

            # ---- attention + output projection, per 512-query block ---------
            out_dr = out_d.rearrange("(a p) n -> p a n", p=128)

            def emit_outproj_chunk(Qprev, so, aTprev, split_dma=False):
                """partial[s, :] = a @ woT for query sub-block so of Qprev."""
                osb = opool.tile([128, D], BF16, tag="osb", name=f"osb{Qprev}{so}")
                for nt in range(2):
                    po = pmm.tile([128, 512], F32, tag="mm")
                    for co in range(2):
                        nc.tensor.matmul(
                            po[:],
                            (aTprev[:, co, so * 128:(so + 1) * 128]),
                            (wo[:, co, nt * 512:(nt + 1) * 512]),
                            start=(co == 0),
                            stop=(co == 1),
                            skip_group_check=True,
                        )
                    nc.vector.tensor_copy(
                        osb[:, nt * 512:(nt + 1) * 512], po[:]
                    )
                    if split_dma:  # tail: drain each half as soon as it's cast
                        nc.sync.dma_start(
                            out_dr[:, Qprev * 4 + so, nt * 512:(nt + 1) * 512],
                            osb[:, nt * 512:(nt + 1) * 512],
                        )
                if not split_dma:
                    nc.sync.dma_start(out_dr[:, Qprev * 4 + so, :], osb[:])

            LAG = 2
            aT_prev = None
            for Q in range(4):
                aT = apool.tile([128, 2, 512], BF16, tag="aT")
                for mo in range(2):
                    nchunks = (Q + 1) * 4
                    out_ps = [
                        psout.tile([65, 512], F32, tag="out", name=f"out_ps{_h}")
                        for _h in range(2)
                    ]
                    exs = {}

                    def chunk_lo(jc):
                        # diagonal chunks only see queries >= (jc-4Q)*128
                        return (jc - 4 * Q) * 128 if jc >= 4 * Q else 0

                    for jc in range(nchunks + LAG):
                        if jc < nchunks:
                            lo = chunk_lo(jc)
                            for hp in range(2):
                                sc = pmm.tile([128, 512], F32, tag="mm")
                                nc.tensor.matmul(
                                    sc[:, lo:512],
                                    (kts[mo][jc // 8][hp * 64:(hp + 1) * 64,
                                           (jc % 8) * 128:(jc % 8 + 1) * 128]),
                                    (qts[mo][Q // 2][hp * 64:(hp + 1) * 64,
                                           (Q % 2) * 512 + lo:(Q % 2 + 1) * 512]),
                                    start=True,
                                    stop=True,
                                    skip_group_check=True,
                                )
                                ex = work.tile([128, 512], BF16, tag="exp")
                                nc.scalar.activation(
                                    ex[:, lo:512], sc[:, lo:512], EXP, scale=SCALE
                                )
                                if lo > 0 or jc == 4 * Q:
                                    # triangular mask on the 128-col diag band
                                    nc.vector.tensor_mul(
                                        ex[:, lo:lo + 128],
                                        ex[:, lo:lo + 128],
                                        maskt[:],
                                    )
                                exs[(jc, hp)] = ex
                            # interleave previous block's output projection:
                            # two sub-blocks per mo group, at jc>=2 so they
                            # never head-of-line block this group's scores
                            if 2 <= jc < 4 and aT_prev is not None:
                                emit_outproj_chunk(Q - 1, 2 * mo + jc - 2, aT_prev)
                        jd = jc - LAG
                        if jd >= 0:
                            lo = chunk_lo(jd)
                            for hp in range(2):
                                h = 2 * mo + hp
                                nc.tensor.matmul(
                                    out_ps[hp][:, lo:512],
                                    (vts[jd][:, h * 65:(h + 1) * 65]),
                                    (exs.pop((jd, hp))[:, lo:512]),
                                    start=(jd == 0),
                                    stop=(jd == nchunks - 1),
                                    skip_group_check=True,
                                )
                    last_group = (Q == 3 and mo == 1)
                    for hp in range(2):
                        # stage the denominator (psum row 64) to partition 0
                        # on the Scalar engine, which is idle at group ends —
                        # single-partition DVE ops cost ~600ns, so keep them
                        # off the Vector queue where they'd serialize
                        den = rpool.tile([1, 512], F32, tag="den")
                        nc.scalar.activation(
                            den[:], out_ps[hp][64:65, :],
                            mybir.ActivationFunctionType.Copy,
                        )
                        rd_f = rpool.tile([1, 512], F32, tag="rdf")
                        nc.vector.reciprocal_approx_fast(out=rd_f[:], in_=den[:])
                        rd_b = rpool.tile([1, 512], BF16, tag="rdb16")
                        nc.scalar.activation(
                            rd_b[:], rd_f[:], mybir.ActivationFunctionType.Copy
                        )
                        rdb = rpool.tile([64, 512], BF16, tag="rdb")
                        if last_group:
                            # nothing left to feed the PE here: broadcast via
                            # a K=1 matmul (keeps the PE at its ramped
                            # p-state for the final out-proj) and stage the
                            # result to SBUF on the idle Scalar engine
                            rdb_ps = pmm.tile([64, 512], F32, tag="mm")
                            nc.tensor.matmul(
                                rdb_ps[:], (ones64[:]), (rd_b[:]),
                                start=True, stop=True, skip_group_check=True,
                            )
                            nc.scalar.activation(
                                rdb[:], rdb_ps[:],
                                mybir.ActivationFunctionType.Copy,
                            )
                        else:
                            # interior groups: broadcast on GpSimd so the
                            # saturated PE never sees a K=1 matmul
                            nc.gpsimd.partition_broadcast(rdb[:], rd_b[:])
                        nc.vector.tensor_mul(
                            aT[hp * 64:(hp + 1) * 64, mo, :],
                            out_ps[hp][0:64, :],
                            rdb[:],
                        )
                aT_prev = aT

            for so in range(4):  # last block's out-proj (nothing to hide it under)
                emit_outproj_chunk(3, so, aT_prev, split_dma=True)

    nc.compile()
    return nc


_NC = None


def _get_nc():
    global _NC
    if _NC is None:
        _NC = build_bass()
    return _NC


def _causal_mask():
    j = np.arange(128)[:, None]
    i = np.arange(128)[None, :]
    return (j <= i).astype(NP_BF16)


def make_in_maps(in_features, Wq, Wk, Wv, Wo):
    x = np.asarray(in_features, np.float32)
    Wq = np.asarray(Wq, np.float32)
    Wk = np.asarray(Wk, np.float32)
    Wv = np.asarray(Wv, np.float32)
    Wo = np.asarray(Wo, np.float32)
    mask = _causal_mask()

    def pack_x(xb):
        # [S, D] -> [128p, 4quarter, 8ko, 512s]: one contiguous 8KB line
        # per (partition, quarter)
        return np.ascontiguousarray(
            xb.reshape(4, 512, 8, 128).transpose(3, 0, 2, 1)
        ).astype(NP_BF16)

    def pack_w(wT):
        # [D, GC] -> [128p, 8ko, GC]: 4KB per-partition lines
        return np.ascontiguousarray(
            wT.reshape(8, 128, GC).transpose(1, 0, 2)
        ).astype(NP_BF16)

    in_maps = []
    for c in range(N_CORES):
        b, g = divmod(c, 4)
        cols = slice(g * GC, (g + 1) * GC)
        in_maps.append({
            "xT": pack_x(x[b]),
            "wqT": pack_w(Wq.T[:, cols]),
            "wkT": pack_w(Wk.T[:, cols]),
            "wvT": pack_w(Wv.T[:, cols]),
            "woT": np.ascontiguousarray(
                Wo[:, cols].T.reshape(2, 128, D).transpose(1, 0, 2)
            ).astype(NP_BF16),
            "mask": mask,
        })
    return in_maps


def kernel(in_features, Wq, Wk, Wv, Wo):
    global LAST_RESULTS
    nc = _get_nc()
    in_maps = make_in_maps(in_features, Wq, Wk, Wv, Wo)

    res = bass_utils.run_bass_kernel_spmd(
        nc, in_maps, core_ids=list(range(N_CORES)), trace=TRACE,
    )
    LAST_RESULTS = res
    parts = [res.results[c]["out"].astype(np.float32) for c in range(N_CORES)]
    out = np.stack([
        parts[4 * b] + parts[4 * b + 1] + parts[4 * b + 2] + parts[4 * b + 3]
        for b in range(B)
    ]).astype(np.float32)
    return out


# revision 45
# speedup vs baseline: 1.1749x; 1.1749x over previous
"""Multi-head self-attention (B=2, S=2048, D=1024, H=16, causal) on 8 TRN2 cores.

Sharding: core c handles batch b=c//4 and head-group g=c%4 (4 heads each).
Host pre-transposes x and the weight slices so the kernel never needs an
on-chip transpose, and pre-converts them to bf16 (PE streams run at
1 cycle/row in bf16; DMA halves):
  xT   [1024, 2048] = x[b].T
  wqT/wkT/wvT [1024, 256] = W.T[:, g*256:(g+1)*256]
  woT  [256, 1024] = Wo[:, g*256:(g+1)*256].T
The kernel writes bf16 partial outputs; host sums the 4 per-group partials
per batch in fp32 at the end.

On-chip dataflow per core:
  qT/kT [256, 2048] (head dim on partitions), v [2048, 4*65] (with a ones
  column appended per head so the PV matmul also accumulates the softmax
  denominator in psum row 64).  Scores are computed transposed
  (scoresT[j, i]) so softmax needs no transpose at all; no max-subtraction
  (scores are O(+-6), exp is safe in fp32).

Performance notes (the PE tensor engine only reaches its ramped 2.4 GHz
p-state after ~3us of *continuous* work; any idle resets it to 1.2 GHz, so
everything is built around never letting the PE wait):
  - Attention runs a lag-2 software pipeline: scores(jc) are emitted two
    chunks ahead of PV(jc), so the PE has ~1.3us of score matmuls in its
    queue while the Scalar engine exponentiates a chunk.
  - Causal trimming: for the 4 diagonal j-chunks of each query block the
    scores / exp / PV are restricted to the query range [o*128, 512) that
    can actually attend to that chunk; the causal mask reduces to a single
    [128,128] lower-triangular multiply per diagonal chunk.
  - All PSUM tiles are one 2KB bank: a 6-deep "mm" pool (scores, QKV
    projections, out-proj) + 2 PV accumulators.
  - The output projection of block Q-1 is interleaved two sub-blocks at a
    time into the starts of block Q's two head-pair groups, filling the
    PE while the softmax denominators of the previous group are applied.
  - The softmax 1/denominator is broadcast across the 64 head-dim
    partitions on the (otherwise idle) GpSimd engine.
  - xT is DMAed in four column-quarters and the first projection group
    only needs the first quarter; DMA descriptor generation (~0.6us per
    transfer, serial per issuing queue) is kept off the critical path.
"""

import os
import sys

sys.path.insert(0, "/opt/trn_rl_repo")
os.environ.setdefault("MYCRO_LOCAL_CACHE", "1")

import numpy as np
import ml_dtypes

import concourse.bacc as bacc
import concourse.bass as bass
import concourse.mybir as mybir
import concourse.tile as tile
from concourse import bass_utils

# The agent image's antenv lacks axon_hooks, so bass_utils' trace path dies on
# import.  Register a shim module that lazily builds the ctypes NTFF hook.
if "antenv.axon_hooks" not in sys.modules:
    import types

    _shim = types.ModuleType("antenv.axon_hooks")
    _shim._HOOK = None

    def _set_hook(hook, _m=_shim):
        _m._HOOK = hook

    def _get_hook(_m=_shim):
        if _m._HOOK is None:
            try:
                from trn_agent_boot.trn_boot import _ntff_profile_via_ctypes

                _m._HOOK = _ntff_profile_via_ctypes("/opt/axon/libaxon_pjrt.so")
            except Exception:
                _m._HOOK = None
        return _m._HOOK

    _shim.set_axon_ntff_profile_hook = _set_hook
    _shim.get_axon_ntff_profile_hook = _get_hook
    sys.modules["antenv.axon_hooks"] = _shim

B, S, D, H = 2, 2048, 1024, 16
DK = 64                      # head dim
HC = 4                       # heads per core
GC = HC * DK                 # 256 cols per head-group
N_CORES = 8
SCALE = 1.0 / np.sqrt(DK)    # 0.125

F32 = mybir.dt.float32
BF16 = mybir.dt.bfloat16
NP_BF16 = ml_dtypes.bfloat16

TRACE = False
LAST_RESULTS = None


def build_bass():
    nc = bacc.Bacc("TRN2", target_bir_lowering=False, debug=False)

    # host-repacked layouts: partition-major with long contiguous lines
    # (8KB for x, 4KB for weights) — short (<=1KB) per-partition DMA lines
    # halve the effective HBM bandwidth
    xT_d = nc.dram_tensor("xT", [128, 4, 8, 512], BF16, kind="ExternalInput")
    wqT_d = nc.dram_tensor("wqT", [128, 8, GC], BF16, kind="ExternalInput")
    wkT_d = nc.dram_tensor("wkT", [128, 8, GC], BF16, kind="ExternalInput")
    wvT_d = nc.dram_tensor("wvT", [128, 8, GC], BF16, kind="ExternalInput")
    woT_d = nc.dram_tensor("woT", [128, 2, D], BF16, kind="ExternalInput")
    mask_d = nc.dram_tensor("mask", [128, 128], BF16, kind="ExternalInput")
    out_d = nc.dram_tensor("out", [S, D], BF16, kind="ExternalOutput")

    EXP = mybir.ActivationFunctionType.Exp

    with tile.TileContext(nc) as tc:
        with (
            nc.allow_low_precision(reason="bf16 matmuls, fp32 psum accumulate"),
            tc.tile_pool(name="const", bufs=1) as const,
            tc.tile_pool(name="work", bufs=6) as work,
            tc.tile_pool(name="apool", bufs=2) as apool,
            tc.tile_pool(name="opool", bufs=2) as opool,
            tc.tile_pool(name="rpool", bufs=2) as rpool,
            tc.tile_pool(name="pmm", bufs=4, space="PSUM") as pmm,
            tc.tile_pool(name="psout", bufs=4, space="PSUM") as psout,
        ):
            # ---- load inputs -------------------------------------------------
            # one tile per s-quarter with exactly one DMA each: a reader of
            # quarter q then only waits for quarter q's transfer (a single
            # shared tile would make every reader wait for the whole 4MB)
            xqs = [const.tile([128, 8, 512], BF16, name=f"xq{q}")
                   for q in range(4)]
            # Concurrent DMAs share HBM bandwidth (completion is NOT FIFO),
            # so only the first projection wave's data (wq+wk+wv+quarter 0,
            # ~1.4MB) is put in flight up front.  Quarters 1-3 are released
            # from the scalar queue behind a dummy op that depends on the
            # previous wave's first result, keeping each quarter's transfer
            # nearly exclusive while the PE computes the wave before it.
            nc.sync.dma_start(xqs[0][:], xT_d[:, 0])
            wq = const.tile([128, 8, GC], BF16)
            nc.sync.dma_start(wq[:], wqT_d[:])
            wk = const.tile([128, 8, GC], BF16)
            nc.gpsimd.dma_start(wk[:], wkT_d[:])
            wv = const.tile([128, 8, GC], BF16)
            nc.gpsimd.dma_start(wv[:], wvT_d[:])
            wo = const.tile([128, 2, D], BF16)
            maskt = const.tile([128, 128], BF16)

            ones_b = const.tile([128, 64], BF16)
            nc.vector.memset(ones_b[:], 1.0)
            ones64 = const.tile([1, 64], BF16)
            nc.vector.tensor_copy(ones64[:], ones_b[0:1, :])

            # ---- projections -------------------------------------------------
            # qT/kT: per (head-pair mo, s-half sbh) tiles [128, 1024] so the
            # attention phase can start before all projections finish
            qts = [[const.tile([128, 1024], BF16, name=f"q{m}{s}")
                    for s in range(2)] for m in range(2)]
            kts = [[const.tile([128, 1024], BF16, name=f"k{m}{s}")
                    for s in range(2)] for m in range(2)]
            # v: per j-chunk tiles; per head: 64 value cols + 1 ones col
            vts = []
            for io in range(16):
                vt = const.tile([128, HC * 65], BF16, name=f"v{io}")
                nc.vector.tensor_copy(
                    vt.rearrange("p (h u) -> p h u", u=65)[:, :, 64],
                    ones_b[:, 0:4],
                )
                vts.append(vt)

            # s-quarter outer: each quarter's groups (q, k, and v) only gate
            # on that quarter's slice of the xT DMA, so the PE paces along
            # right behind the input stream
            for sb in range(4):
                for w_sb, dst in ((wq, qts), (wk, kts)):
                    for mo in range(2):
                        ps = pmm.tile([128, 512], F32, tag="mm")
                        for ko in range(8):
                            nc.tensor.matmul(
                                ps[:],
                                (w_sb[:, ko, mo * 128:(mo + 1) * 128]),
                                (xqs[sb][:, ko, :]),
                                start=(ko == 0),
                                stop=(ko == 7),
                                skip_group_check=True,
                            )
                        nc.vector.tensor_copy(
                            dst[mo][sb // 2][:, (sb % 2) * 512:(sb % 2 + 1) * 512],
                            ps[:],
                        )
                        if w_sb is wq and mo == 0 and sb < 3:
                            # release the next quarter only once this wave's
                            # first copy lands: the tiny copy into the target
                            # tile gives the deferred DMA a write-after-write
                            # dependency the scheduler cannot hoist past
                            nc.vector.tensor_copy(
                                xqs[sb + 1][0:1, 0:1, 0:8],
                                dst[mo][sb // 2][0:1, (sb % 2) * 512:
                                                 (sb % 2) * 512 + 8],
                            )
                            nc.sync.dma_start(
                                xqs[sb + 1][:], xT_d[:, sb + 1]
                            )
                            if sb == 0:
                                nc.sync.dma_start(wo[:], woT_d[:])
                                nc.sync.dma_start(maskt[:], mask_d[:])
                for io in range(4 * sb, 4 * sb + 4):
                    ps = pmm.tile([128, 256], F32, tag="mm")
                    for ko in range(8):
                        nc.tensor.matmul(
                            ps[:],
                            (xqs[sb][:, ko, (io % 4) * 128:(io % 4 + 1) * 128]),
                            (wv[:, ko, :]),
                            start=(ko == 0),
                            stop=(ko == 7),
                        )
                    nc.vector.tensor_copy(
                        vts[io].rearrange("p (h u) -> p h u", u=65)[:, :, 0:64],
                        ps.rearrange("p (h e) -> p h e", e=64),
                    )

            # ---- attention + output projection, per 512-query block ---------
            out_dr = out_d.rearrange("(a p) n -> p a n", p=128)

            def emit_outproj_chunk(Qprev, so, aTprev, split_dma=False):
                """partial[s, :] = a @ woT for query sub-block so of Qprev."""
                osb = opool.tile([128, D], BF16, tag="osb", name=f"osb{Qprev}{so}")
                for nt in range(2):
                    po = pmm.tile([128, 512], F32, tag="mm")
                    for co in range(2):
                        nc.tensor.matmul(
                            po[:],
                            (aTprev[:, co, so * 128:(so + 1) * 128]),
                            (wo[:, co, nt * 512:(nt + 1) * 512]),
                            start=(co == 0),
                            stop=(co == 1),
                            skip_group_check=True,
                        )
                    nc.vector.tensor_copy(
                        osb[:, nt * 512:(nt + 1) * 512], po[:]
                    )
                    if split_dma:  # tail: drain each half as soon as it's cast
                        nc.sync.dma_start(
                            out_dr[:, Qprev * 4 + so, nt * 512:(nt + 1) * 512],
                            osb[:, nt * 512:(nt + 1) * 512],
                        )
                if not split_dma:
                    nc.sync.dma_start(out_dr[:, Qprev * 4 + so, :], osb[:])

            LAG = 2
            aT_prev = None
            for Q in range(4):
                aT = apool.tile([128, 2, 512], BF16, tag="aT")
                for mo in range(2):
                    nchunks = (Q + 1) * 4
                    out_ps = [
                        psout.tile([65, 512], F32, tag="out", name=f"out_ps{_h}")
                        for _h in range(2)
                    ]
                    exs = {}

                    def chunk_lo(jc):
                        # diagonal chunks only see queries >= (jc-4Q)*128
                        return (jc - 4 * Q) * 128 if jc >= 4 * Q else 0

                    for jc in range(nchunks + LAG):
                        if jc < nchunks:
                            lo = chunk_lo(jc)
                            for hp in range(2):
                                sc = pmm.tile([128, 512], F32, tag="mm")
                                nc.tensor.matmul(
                                    sc[:, lo:512],
                                    (kts[mo][jc // 8][hp * 64:(hp + 1) * 64,
                                           (jc % 8) * 128:(jc % 8 + 1) * 128]),
                                    (qts[mo][Q // 2][hp * 64:(hp + 1) * 64,
                                           (Q % 2) * 512 + lo:(Q % 2 + 1) * 512]),
                                    start=True,
                                    stop=True,
                                    skip_group_check=True,
                                )
                                ex = work.tile([128, 512], BF16, tag="exp")
                                nc.scalar.activation(
                                    ex[:, lo:512], sc[:, lo:512], EXP, scale=SCALE
                                )
                                if lo > 0 or jc == 4 * Q:
                                    # triangular mask on the 128-col diag band
                                    nc.vector.tensor_mul(
                                        ex[:, lo:lo + 128],
                                        ex[:, lo:lo + 128],
                                        maskt[:],
                                    )
                                exs[(jc, hp)] = ex
                            # interleave previous block's output projection:
                            # two sub-blocks per mo group, at jc>=2 so they
                            # never head-of-line block this group's scores
                            if 2 <= jc < 4 and aT_prev is not None:
                                emit_outproj_chunk(Q - 1, 2 * mo + jc - 2, aT_prev)
                        jd = jc - LAG
                        if jd >= 0:
                            lo = chunk_lo(jd)
                            for hp in range(2):
                                h = 2 * mo + hp
                                nc.tensor.matmul(
                                    out_ps[hp][:, lo:512],
                                    (vts[jd][:, h * 65:(h + 1) * 65]),
                                    (exs.pop((jd, hp))[:, lo:512]),
                                    start=(jd == 0),
                                    stop=(jd == nchunks - 1),
                                    skip_group_check=True,
                                )
                    last_group = (Q == 3 and mo == 1)
                    for hp in range(2):
                        # the staging copy and cast run on Vector for
                        # interior groups (scalar ops here would queue ahead
                        # of the next group's EXPs and stall the attention
                        # pipeline); only the final group, whose scalar queue
                        # is drained, moves them to the idle Scalar engine
                        cp_eng = nc.scalar if last_group else nc.vector
                        den = rpool.tile([1, 512], F32, tag="den")
                        if last_group:
                            nc.scalar.activation(
                                den[:], out_ps[hp][64:65, :],
                                mybir.ActivationFunctionType.Copy,
                            )
                        else:
                            nc.vector.tensor_copy(den[:], out_ps[hp][64:65, :])
                        rd_f = rpool.tile([1, 512], F32, tag="rdf")
                        nc.vector.reciprocal_approx_fast(out=rd_f[:], in_=den[:])
                        rd_b = rpool.tile([1, 512], BF16, tag="rdb16")
                        if last_group:
                            nc.scalar.activation(
                                rd_b[:], rd_f[:],
                                mybir.ActivationFunctionType.Copy,
                            )
                        else:
                            nc.vector.tensor_copy(rd_b[:], rd_f[:])
                        rdb = rpool.tile([64, 512], BF16, tag="rdb")
                        if last_group:
                            # nothing left to feed the PE here: broadcast via
                            # a K=1 matmul (keeps the PE at its ramped
                            # p-state for the final out-proj) and stage the
                            # result to SBUF on the idle Scalar engine
                            rdb_ps = pmm.tile([64, 512], F32, tag="mm")
                            nc.tensor.matmul(
                                rdb_ps[:], (ones64[:]), (rd_b[:]),
                                start=True, stop=True, skip_group_check=True,
                            )
                            nc.scalar.activation(
                                rdb[:], rdb_ps[:],
                                mybir.ActivationFunctionType.Copy,
                            )
                        else:
                            # interior groups: broadcast on GpSimd so the
                            # saturated PE never sees a K=1 matmul
                            nc.gpsimd.partition_broadcast(rdb[:], rd_b[:])
                        nc.vector.tensor_mul(
                            aT[hp * 64:(hp + 1) * 64, mo, :],
                            out_ps[hp][0:64, :],
                            rdb[:],
                        )
                aT_prev = aT

            for so in range(4):  # last block's out-proj (nothing to hide it under)
                emit_outproj_chunk(3, so, aT_prev, split_dma=True)

    nc.compile()
    return nc


_NC = None


def _get_nc():
    global _NC
    if _NC is None:
        _NC = build_bass()
    return _NC


def _causal_mask():
    j = np.arange(128)[:, None]
    i = np.arange(128)[None, :]
    return (j <= i).astype(NP_BF16)


def make_in_maps(in_features, Wq, Wk, Wv, Wo):
    x = np.asarray(in_features, np.float32)
    Wq = np.asarray(Wq, np.float32)
    Wk = np.asarray(Wk, np.float32)
    Wv = np.asarray(Wv, np.float32)
    Wo = np.asarray(Wo, np.float32)
    mask = _causal_mask()

    def pack_x(xb):
        # [S, D] -> [128p, 4quarter, 8ko, 512s]: one contiguous 8KB line
        # per (partition, quarter)
        return np.ascontiguousarray(
            xb.reshape(4, 512, 8, 128).transpose(3, 0, 2, 1)
        ).astype(NP_BF16)

    def pack_w(wT):
        # [D, GC] -> [128p, 8ko, GC]: 4KB per-partition lines
        return np.ascontiguousarray(
            wT.reshape(8, 128, GC).transpose(1, 0, 2)
        ).astype(NP_BF16)

    in_maps = []
    for c in range(N_CORES):
        b, g = divmod(c, 4)
        cols = slice(g * GC, (g + 1) * GC)
        in_maps.append({
            "xT": pack_x(x[b]),
            "wqT": pack_w(Wq.T[:, cols]),
            "wkT": pack_w(Wk.T[:, cols]),
            "wvT": pack_w(Wv.T[:, cols]),
            "woT": np.ascontiguousarray(
                Wo[:, cols].T.reshape(2, 128, D).transpose(1, 0, 2)
            ).astype(NP_BF16),
            "mask": mask,
        })
    return in_maps


def kernel(in_features, Wq, Wk, Wv, Wo):
    global LAST_RESULTS
    nc = _get_nc()
    in_maps = make_in_maps(in_features, Wq, Wk, Wv, Wo)

    res = bass_utils.run_bass_kernel_spmd(
        nc, in_maps, core_ids=list(range(N_CORES)), trace=TRACE,
    )
    LAST_RESULTS = res
    parts = [res.results[c]["out"].astype(np.float32) for c in range(N_CORES)]
    out = np.stack([
        parts[4 * b] + parts[4 * b + 1] + parts[4 * b + 2] + parts[4 * b + 3]
        for b in range(B)
    ]).astype(np.float32)
    return out


# revision 75
# speedup vs baseline: 1.2487x; 1.0628x over previous
"""Multi-head self-attention (B=2, S=2048, D=1024, H=16, causal) on 8 TRN2 cores.

Sharding: core c handles batch b=c//4 and head-group g=c%4 (4 heads each).
Host pre-transposes x and the weight slices so the kernel never needs an
on-chip transpose, and pre-converts them to bf16 (PE streams run at
1 cycle/row in bf16; DMA halves):
  xT   [1024, 2048] = x[b].T
  wqT/wkT/wvT [1024, 256] = W.T[:, g*256:(g+1)*256]
  woT  [256, 1024] = Wo[:, g*256:(g+1)*256].T
The kernel writes bf16 partial outputs; host sums the 4 per-group partials
per batch in fp32 at the end.

On-chip dataflow per core:
  qT/kT [256, 2048] (head dim on partitions), v [2048, 4*65] (with a ones
  column appended per head so the PV matmul also accumulates the softmax
  denominator in psum row 64).  Scores are computed transposed
  (scoresT[j, i]) so softmax needs no transpose at all; no max-subtraction
  (scores are O(+-6), exp is safe in fp32).

Performance notes (the PE tensor engine only reaches its ramped 2.4 GHz
p-state after ~3us of *continuous* work; any idle resets it to 1.2 GHz, so
everything is built around never letting any engine wait):
  - The QKV projections are computed in four s-quarter waves; waves 1-3
    are drip-fed group-by-group into the previous attention block's chunk
    loop so the PE computes them inside the slack where it would otherwise
    wait on the Scalar engine's EXP stream (the attention bottleneck).
  - Attention runs a lag-2 software pipeline: scores(jc) are emitted two
    chunks ahead of PV(jc), so the PE has score matmuls queued while the
    Scalar engine exponentiates a chunk.
  - Causal trimming: for the 4 diagonal j-chunks of each query block the
    scores / exp / PV are restricted to the query range [o*128, 512) that
    can actually attend to that chunk; the causal mask reduces to a single
    [128,128] lower-triangular multiply per diagonal chunk.
  - PSUM: 4 one-bank "mm" buffers (scores / projections / out-proj) + 4
    one-bank PV accumulators, so consecutive (Q, head-pair) groups
    double-buffer and PV never waits on the previous group's normalize.
  - The output projection of block Q-1 is interleaved into block Q at
    chunks 4-5, late enough that the psum-ring coupling to the previous
    normalize never stalls the PE.
  - The softmax 1/denominator is broadcast across the 64 head-dim
    partitions on the (otherwise idle) GpSimd engine; the final group
    instead uses a K=1 matmul broadcast + Scalar-engine staging, which
    doubles as a p-state warmer for the tail out-proj.
  - Input DMAs form a serial chain (each released by a corner-copy WAW
    dependency when the previous quarter lands): concurrent DMAs
    fair-share HBM bandwidth, so chaining keeps each transfer exclusive
    and the first matmul gates on only ~1MB.  Host-side repacking gives
    every transfer >=4KB contiguous per-partition lines (short lines halve
    effective DMA bandwidth).
"""

import os
import sys

sys.path.insert(0, "/opt/trn_rl_repo")
os.environ.setdefault("MYCRO_LOCAL_CACHE", "1")

import numpy as np
import ml_dtypes

import concourse.bacc as bacc
import concourse.bass as bass
import concourse.mybir as mybir
import concourse.tile as tile
from concourse import bass_utils

# The agent image's antenv lacks axon_hooks, so bass_utils' trace path dies on
# import.  Register a shim module that lazily builds the ctypes NTFF hook.
if "antenv.axon_hooks" not in sys.modules:
    import types

    _shim = types.ModuleType("antenv.axon_hooks")
    _shim._HOOK = None

    def _set_hook(hook, _m=_shim):
        _m._HOOK = hook

    def _get_hook(_m=_shim):
        if _m._HOOK is None:
            try:
                from trn_agent_boot.trn_boot import _ntff_profile_via_ctypes

                _m._HOOK = _ntff_profile_via_ctypes("/opt/axon/libaxon_pjrt.so")
            except Exception:
                _m._HOOK = None
        return _m._HOOK

    _shim.set_axon_ntff_profile_hook = _set_hook
    _shim.get_axon_ntff_profile_hook = _get_hook
    sys.modules["antenv.axon_hooks"] = _shim

B, S, D, H = 2, 2048, 1024, 16
DK = 64                      # head dim
HC = 4                       # heads per core
GC = HC * DK                 # 256 cols per head-group
N_CORES = 8
SCALE = 1.0 / np.sqrt(DK)    # 0.125

F32 = mybir.dt.float32
BF16 = mybir.dt.bfloat16
NP_BF16 = ml_dtypes.bfloat16

TRACE = False
LAST_RESULTS = None


def build_bass():
    nc = bacc.Bacc("TRN2", target_bir_lowering=False, debug=False)

    # host-repacked layouts: partition-major with long contiguous lines
    # (8KB for x, 4KB for weights) — short (<=1KB) per-partition DMA lines
    # halve the effective HBM bandwidth
    xT_d = nc.dram_tensor("xT", [128, 4, 8, 512], BF16, kind="ExternalInput")
    wqT_d = nc.dram_tensor("wqT", [128, 8, GC], BF16, kind="ExternalInput")
    wkT_d = nc.dram_tensor("wkT", [128, 8, GC], BF16, kind="ExternalInput")
    wvT_d = nc.dram_tensor("wvT", [128, 8, GC], BF16, kind="ExternalInput")
    woT_d = nc.dram_tensor("woT", [128, 2, D], BF16, kind="ExternalInput")
    mask_d = nc.dram_tensor("mask", [128, 128], BF16, kind="ExternalInput")
    out_d = nc.dram_tensor("out", [S, D], BF16, kind="ExternalOutput")

    EXP = mybir.ActivationFunctionType.Exp

    with tile.TileContext(nc) as tc:
        with (
            nc.allow_low_precision(reason="bf16 matmuls, fp32 psum accumulate"),
            tc.tile_pool(name="const", bufs=1) as const,
            tc.tile_pool(name="work", bufs=6) as work,
            tc.tile_pool(name="apool", bufs=2) as apool,
            tc.tile_pool(name="opool", bufs=2) as opool,
            tc.tile_pool(name="rpool", bufs=2) as rpool,
            tc.tile_pool(name="pmm", bufs=4, space="PSUM") as pmm,
            tc.tile_pool(name="psout", bufs=4, space="PSUM") as psout,
        ):
            # ---- load inputs -------------------------------------------------
            # one tile per s-quarter with exactly one DMA each: a reader of
            # quarter q then only waits for quarter q's transfer (a single
            # shared tile would make every reader wait for the whole 4MB).
            # quarter 0 is further split in two ko-halves so the very first
            # projection matmuls start after ~0.5MB of input
            xq0a = const.tile([128, 4, 512], BF16, name="xq0a")
            xq0b = const.tile([128, 4, 512], BF16, name="xq0b")
            xqs = [None] + [const.tile([128, 8, 512], BF16, name=f"xq{q}")
                            for q in range(1, 4)]

            def xap(sb, ko):
                if sb == 0:
                    t = xq0a if ko < 4 else xq0b
                    return t[:, ko % 4, :]
                return xqs[sb][:, ko, :]
            # Concurrent DMAs share HBM bandwidth (completion is NOT FIFO),
            # so only the first projection wave's data (wq+wk+wv+quarter 0,
            # ~1.4MB) is put in flight up front.  Quarters 1-3 are released
            # from the scalar queue behind a dummy op that depends on the
            # previous wave's first result, keeping each quarter's transfer
            # nearly exclusive while the PE computes the wave before it.
            nc.sync.dma_start(xq0a[:], xT_d[:, 0, 0:4])
            wq = const.tile([128, 8, GC], BF16)
            nc.sync.dma_start(wq[:], wqT_d[:])
            wk = const.tile([128, 8, GC], BF16)
            nc.gpsimd.dma_start(wk[:], wkT_d[:])
            wv = const.tile([128, 8, GC], BF16)
            nc.gpsimd.dma_start(wv[:], wvT_d[:])
            maskt = const.tile([128, 128], BF16)
            wo = const.tile([128, 2, D], BF16)

            # serial DMA chain: each transfer is released (via a tiny
            # write-after-write corner copy the scheduler cannot hoist past)
            # when the previous quarter lands, so transfers stay nearly
            # exclusive instead of fair-sharing HBM bandwidth
            def chain_dma(trigger_corner, dst_tile, dst_corner, dram_ap):
                nc.vector.tensor_copy(dst_corner, trigger_corner)
                nc.sync.dma_start(dst_tile[:], dram_ap)

            chain_dma(xq0a[0:1, 0:1, 0:8], xq0b, xq0b[0:1, 0:1, 0:8],
                      xT_d[:, 0, 4:8])
            chain_dma(xq0b[0:1, 0:1, 0:8], xqs[1], xqs[1][0:1, 0:1, 0:8],
                      xT_d[:, 1])
            chain_dma(xqs[1][0:1, 0:1, 0:8], maskt, maskt[0:1, 0:8], mask_d[:])
            chain_dma(xqs[1][0:1, 0:1, 0:8], wo, wo[0:1, 0:1, 0:8], woT_d[:])
            chain_dma(xqs[1][0:1, 0:1, 0:8], xqs[2], xqs[2][0:1, 0:1, 0:8],
                      xT_d[:, 2])
            chain_dma(xqs[2][0:1, 0:1, 0:8], xqs[3], xqs[3][0:1, 0:1, 0:8],
                      xT_d[:, 3])

            ones_b = const.tile([128, 64], BF16)
            nc.vector.memset(ones_b[:], 1.0)
            ones64 = const.tile([1, 64], BF16)
            nc.vector.tensor_copy(ones64[:], ones_b[0:1, :])

            # ---- projections -------------------------------------------------
            # qT/kT: per (head-pair mo, s-quarter) tiles [128, 512] so each
            # attention block only depends on the quarters it actually reads
            qts = [[const.tile([128, 512], BF16, name=f"q{m}{s}")
                    for s in range(4)] for m in range(2)]
            kts = [[const.tile([128, 512], BF16, name=f"k{m}{s}")
                    for s in range(4)] for m in range(2)]
            # v: per j-chunk tiles; per head: 64 value cols + 1 ones col
            vts = []
            for io in range(16):
                vt = const.tile([128, HC * 65], BF16, name=f"v{io}")
                nc.vector.tensor_copy(
                    vt.rearrange("p (h u) -> p h u", u=65)[:, :, 64],
                    ones_b[:, 0:4],
                )
                vts.append(vt)

            # ---- attention + output projection, per 512-query block ---------
            out_dr = out_d.rearrange("(a p) n -> p a n", p=128)

            def emit_outproj_chunk(Qprev, so, aTprev, split_dma=False):
                """partial[s, :] = a @ woT for query sub-block so of Qprev."""
                osb = opool.tile([128, D], BF16, tag="osb", name=f"osb{Qprev}{so}")
                for nt in range(2):
                    po = pmm.tile([128, 512], F32, tag="mm")
                    for co in range(2):
                        nc.tensor.matmul(
                            po[:],
                            (aTprev[:, co, so * 128:(so + 1) * 128]),
                            (wo[:, co, nt * 512:(nt + 1) * 512]),
                            start=(co == 0),
                            stop=(co == 1),
                            skip_group_check=True,
                        )
                    if split_dma and nt == 1:
                        # tail: odd halves cast on the (now idle) Scalar
                        # engine so the two casts run in parallel
                        nc.scalar.activation(
                            osb[:, nt * 512:(nt + 1) * 512], po[:],
                            mybir.ActivationFunctionType.Copy,
                        )
                    else:
                        nc.vector.tensor_copy(
                            osb[:, nt * 512:(nt + 1) * 512], po[:]
                        )
                    if split_dma:  # tail: drain each half as soon as it's cast
                        nc.sync.dma_start(
                            out_dr[:, Qprev * 4 + so, nt * 512:(nt + 1) * 512],
                            osb[:, nt * 512:(nt + 1) * 512],
                        )
                if not split_dma:
                    nc.sync.dma_start(out_dr[:, Qprev * 4 + so, :], osb[:])

            def emit_qk_group(sb, w_sb, dst, mo):
                ps = pmm.tile([128, 512], F32, tag="mm")
                for ko in range(8):
                    nc.tensor.matmul(
                        ps[:],
                        (w_sb[:, ko, mo * 128:(mo + 1) * 128]),
                        (xap(sb, ko)),
                        start=(ko == 0),
                        stop=(ko == 7),
                        skip_group_check=True,
                    )
                nc.vector.tensor_copy(dst[mo][sb][:], ps[:])

            def emit_v_group(sb, io):
                ps = pmm.tile([128, 256], F32, tag="mm")
                for ko in range(8):
                    nc.tensor.matmul(
                        ps[:],
                        (xap(sb, ko)[:, (io % 4) * 128:(io % 4 + 1) * 128]),
                        (wv[:, ko, :]),
                        start=(ko == 0),
                        stop=(ko == 7),
                    )
                nc.vector.tensor_copy(
                    vts[io].rearrange("p (h u) -> p h u", u=65)[:, :, 0:64],
                    ps.rearrange("p (h e) -> p h e", e=64),
                )

            def proj_wave(sb):
                return ([lambda w=w, d=d, m=m: emit_qk_group(sb, w, d, m)
                         for w, d in ((wq, qts), (wk, kts)) for m in range(2)]
                        + [lambda i=i: emit_v_group(sb, i)
                           for i in range(4 * sb, 4 * sb + 4)])

            LAG = 2
            aT_prev = None
            for g in proj_wave(0):  # wave 0 has nothing to hide under
                g()
            pending_proj = proj_wave(1)
            for Q in range(4):
                # ---- attention block Q -----------------------------------
                # projection wave Q+1 is drip-fed into this block's chunk
                # loop so the PE computes it inside the gaps where it would
                # otherwise wait on the Scalar engine's EXP stream (the
                # attention bottleneck), which keeps EXP continuously fed
                aT = apool.tile([128, 2, 512], BF16, tag="aT")
                for mo in range(2):
                    nchunks = (Q + 1) * 4
                    out_ps = [
                        psout.tile([65, 512], F32, tag="out", name=f"out_ps{_h}")
                        for _h in range(2)
                    ]
                    exs = {}

                    def chunk_lo(jc):
                        # diagonal chunks only see queries >= (jc-4Q)*128
                        return (jc - 4 * Q) * 128 if jc >= 4 * Q else 0

                    for jc in range(nchunks + LAG):
                        if jc < nchunks:
                            lo = chunk_lo(jc)
                            for hp in range(2):
                                sc = pmm.tile([128, 512], F32, tag="mm")
                                nc.tensor.matmul(
                                    sc[:, lo:512],
                                    (kts[mo][jc // 4][hp * 64:(hp + 1) * 64,
                                           (jc % 4) * 128:(jc % 4 + 1) * 128]),
                                    (qts[mo][Q][hp * 64:(hp + 1) * 64, lo:512]),
                                    start=True,
                                    stop=True,
                                    skip_group_check=True,
                                )
                                ex = work.tile([128, 512], BF16, tag="exp")
                                nc.scalar.activation(
                                    ex[:, lo:512], sc[:, lo:512], EXP, scale=SCALE
                                )
                                if lo > 0 or jc == 4 * Q:
                                    # triangular mask on the 128-col diag band
                                    nc.vector.tensor_mul(
                                        ex[:, lo:lo + 128],
                                        ex[:, lo:lo + 128],
                                        maskt[:],
                                    )
                                exs[(jc, hp)] = ex
                            # interleave previous block's output projection:
                            # two sub-blocks per mo group, at jc==4,5: by the
                            # time the psum-ring couples the scores to the
                            # out-proj drain, the previous group's softmax
                            # normalize (which produces aT) has finished
                            if 4 <= jc < 6 and aT_prev is not None:
                                emit_outproj_chunk(
                                    Q - 1, 2 * mo + jc - 4, aT_prev
                                )
                        # drip-feed one projection group of the next wave per
                        # chunk slot (skipping the out-proj slots); the
                        # serial DMA chain has landed its quarter well before
                        # any block's chunk loop reaches here.  Block 2 drips
                        # wave 3 at half pace so the leftovers spill into
                        # block 3's early chunks (otherwise EXP-starved; its
                        # quarter-3 k/v aren't read before chunk 12)
                        if (pending_proj and jc >= 2
                                and not (4 <= jc < 6 and aT_prev is not None)
                                and not (Q == 2 and jc % 2 == 1)):
                            pending_proj.pop(0)()
                        jd = jc - LAG
                        if jd >= 0:
                            lo = chunk_lo(jd)
                            for hp in range(2):
                                h = 2 * mo + hp
                                nc.tensor.matmul(
                                    out_ps[hp][:, lo:512],
                                    (vts[jd][:, h * 65:(h + 1) * 65]),
                                    (exs.pop((jd, hp))[:, lo:512]),
                                    start=(jd == 0),
                                    stop=(jd == nchunks - 1),
                                    skip_group_check=True,
                                )
                    last_group = (Q == 3 and mo == 1)
                    for hp in range(2):
                        # the staging copy and cast run on Vector for
                        # interior groups (scalar ops here would queue ahead
                        # of the next group's EXPs and stall the attention
                        # pipeline); only the final group, whose scalar queue
                        # is drained, moves them to the idle Scalar engine
                        cp_eng = nc.scalar if last_group else nc.vector
                        den = rpool.tile([1, 512], F32, tag="den")
                        if last_group:
                            nc.scalar.activation(
                                den[:], out_ps[hp][64:65, :],
                                mybir.ActivationFunctionType.Copy,
                            )
                        else:
                            nc.vector.tensor_copy(den[:], out_ps[hp][64:65, :])
                        rd_f = rpool.tile([1, 512], F32, tag="rdf")
                        nc.vector.reciprocal_approx_fast(out=rd_f[:], in_=den[:])
                        rd_b = rpool.tile([1, 512], BF16, tag="rdb16")
                        if last_group:
                            nc.scalar.activation(
                                rd_b[:], rd_f[:],
                                mybir.ActivationFunctionType.Copy,
                            )
                        else:
                            nc.vector.tensor_copy(rd_b[:], rd_f[:])
                        rdb = rpool.tile([64, 512], BF16, tag="rdb")
                        if last_group:
                            # nothing left to feed the PE here: broadcast via
                            # a K=1 matmul (keeps the PE at its ramped
                            # p-state for the final out-proj) and stage the
                            # result to SBUF on the idle Scalar engine
                            rdb_ps = pmm.tile([64, 512], F32, tag="mm")
                            nc.tensor.matmul(
                                rdb_ps[:], (ones64[:]), (rd_b[:]),
                                start=True, stop=True, skip_group_check=True,
                            )
                            nc.scalar.activation(
                                rdb[:], rdb_ps[:],
                                mybir.ActivationFunctionType.Copy,
                            )
                        else:
                            # interior groups: broadcast on GpSimd so the
                            # saturated PE never sees a K=1 matmul
                            nc.gpsimd.partition_broadcast(rdb[:], rd_b[:])
                        nc.vector.tensor_mul(
                            aT[hp * 64:(hp + 1) * 64, mo, :],
                            out_ps[hp][0:64, :],
                            rdb[:],
                        )
                aT_prev = aT
                if Q != 2:
                    # block Q+1 needs all of wave Q+1 early: drain leftovers
                    # (wave 3's leftovers instead flow into block 3's slots)
                    for g in pending_proj:
                        g()
                    pending_proj = []
                if Q + 2 <= 3:
                    pending_proj = pending_proj + proj_wave(Q + 2)

            for so in range(4):  # last block's out-proj (nothing to hide it under)
                emit_outproj_chunk(3, so, aT_prev, split_dma=True)

    nc.compile()
    return nc


_NC = None


def _get_nc():
    global _NC
    if _NC is None:
        _NC = build_bass()
    return _NC


def _causal_mask():
    j = np.arange(128)[:, None]
    i = np.arange(128)[None, :]
    return (j <= i).astype(NP_BF16)


def make_in_maps(in_features, Wq, Wk, Wv, Wo):
    x = np.asarray(in_features, np.float32)
    Wq = np.asarray(Wq, np.float32)
    Wk = np.asarray(Wk, np.float32)
    Wv = np.asarray(Wv, np.float32)
    Wo = np.asarray(Wo, np.float32)
    mask = _causal_mask()

    def pack_x(xb):
        # [S, D] -> [128p, 4quarter, 8ko, 512s]: one contiguous 8KB line
        # per (partition, quarter)
        return np.ascontiguousarray(
            xb.reshape(4, 512, 8, 128).transpose(3, 0, 2, 1)
        ).astype(NP_BF16)

    def pack_w(wT):
        # [D, GC] -> [128p, 8ko, GC]: 4KB per-partition lines
        return np.ascontiguousarray(
            wT.reshape(8, 128, GC).transpose(1, 0, 2)
        ).astype(NP_BF16)

    in_maps = []
    for c in range(N_CORES):
        b, g = divmod(c, 4)
        cols = slice(g * GC, (g + 1) * GC)
        in_maps.append({
            "xT": pack_x(x[b]),
            "wqT": pack_w(Wq.T[:, cols]),
            "wkT": pack_w(Wk.T[:, cols]),
            "wvT": pack_w(Wv.T[:, cols]),
            "woT": np.ascontiguousarray(
                Wo[:, cols].T.reshape(2, 128, D).transpose(1, 0, 2)
            ).astype(NP_BF16),
            "mask": mask,
        })
    return in_maps


def kernel(in_features, Wq, Wk, Wv, Wo):
    global LAST_RESULTS
    nc = _get_nc()
    in_maps = make_in_maps(in_features, Wq, Wk, Wv, Wo)

    res = bass_utils.run_bass_kernel_spmd(
        nc, in_maps, core_ids=list(range(N_CORES)), trace=TRACE,
    )
    LAST_RESULTS = res
    parts = [res.results[c]["out"].astype(np.float32) for c in range(N_CORES)]
    out = np.stack([
        parts[4 * b] + parts[4 * b + 1] + parts[4 * b + 2] + parts[4 * b + 3]
        for b in range(B)
    ]).astype(np.float32)
    return out
